# revision 15
# baseline (speedup 1.0000x reference)
"""Trainium2 Bass kernel for tiled-MoE NeRF MLP (moe_routing).

Strategy:
 - Host: compute per-layer tile indices (pure function of pixel coords),
   globally sort pixels by (idx0, idx1, idx2). Each of the 8 cores gets a
   contiguous 32768-pixel slice = 2 full idx0 expert groups. Within a core,
   idx1 expert runs are 1024 px and idx2 expert runs are 64 px at stride
   1024 -- all compile-time-known strided access patterns.
 - Device (per core, SPMD): feature-major activations [d on partitions,
   pixels on free dim]. Positional encoding via ScalarE Sin (angles are
   range-reduced on host). 3 expert layers + final 256->3 layer as
   float32r matmuls (full fp32 storage, 1 cycle/row at N>=256).
 - Output [3, 32768] per core, host inverse-permutes to [262144, 3].
"""

import os
import sys
import time

import numpy as np

os.environ.setdefault("MYCRO_LOCAL_CACHE", "1")

try:
    import concourse.bass as bass
except ImportError:  # fresh grading dir: repo is in the container, not on path
    sys.path.insert(0, "/opt/trn_rl_repo")
    import concourse.bass as bass

import concourse.mybir as mybir
import concourse.tile as tile
from concourse import bacc
from concourse.bass_utils import run_bass_kernel_spmd

AF = mybir.ActivationFunctionType
ALU = mybir.AluOpType
F32 = mybir.dt.float32
F32R = mybir.dt.float32r

IMG = 512
NPIX = IMG * IMG          # 262144
NCORES = 8
NPC = NPIX // NCORES      # 32768 pixels per core
CHUNK = 512               # pixels per matmul chunk (one PSUM bank of fp32)
BLK = 4096                # pixels per block (4 idx1-runs of one idx0 group)
CPB = BLK // CHUNK        # 8 chunks per block
NBLK_FULL = NPC // BLK    # 8 blocks per core
NFREQ = 13
DH = 256
NEXP = 16

LAST_RESULT = None        # BassKernelResults of most recent run (for profiling)
LAST_EXEC_S = None        # wall time of the device execute (incl. PJRT dispatch)
_NC_CACHE = {}


def _r(ap):
    return ap.bitcast(F32R)


def _build_nc(nblk=NBLK_FULL):
    """Build the single-core Bass program (SPMD across 8 cores)."""
    npc = nblk * BLK
    nc = bacc.Bacc(None, target_bir_lowering=False)
    axy_d = nc.declare_dram_parameter("axy", [66, npc], F32R, isOutput=False)
    w0_d = nc.declare_dram_parameter("w0e", [66, 512], F32R, isOutput=False)
    w1_d = nc.declare_dram_parameter("w1e", [128, NEXP * 2 * DH], F32R, isOutput=False)
    w2_d = nc.declare_dram_parameter("w2e", [128, NEXP * 2 * DH], F32R, isOutput=False)
    wl_d = nc.declare_dram_parameter("wle", [128, 6], F32R, isOutput=False)
    b0_d = nc.declare_dram_parameter("b0e", [128, 4], F32, isOutput=False)
    b1_d = nc.declare_dram_parameter("b1e", [128, 4], F32, isOutput=False)
    b2_d = nc.declare_dram_parameter("b2e", [128, 4], F32, isOutput=False)
    bl_d = nc.declare_dram_parameter("ble", [3, 1], F32, isOutput=False)
    out_d = nc.declare_dram_parameter("out", [3, npc], F32, isOutput=True)

    with tile.TileContext(nc) as tc:
        with (
            tc.tile_pool(name="const", bufs=1) as cp,
            tc.tile_pool(name="blkp", bufs=2) as bp,
            tc.tile_pool(name="wk", bufs=3) as wp,
            tc.tile_pool(name="ps", bufs=2, space=bass.MemorySpace.PSUM) as pp,
        ):
            def cload(shape, src, tag, dt=F32):
                t = cp.tile(shape, dt, tag=tag, name=tag)
                nc.sync.dma_start(t[:, :], src[:, :])
                return t

            w0t = cload([66, 512], w0_d, "w0t", dt=F32R)
            w1t = cload([128, NEXP * 2 * DH], w1_d, "w1t", dt=F32R)
            w2t = cload([128, NEXP * 2 * DH], w2_d, "w2t", dt=F32R)
            wlt = cload([128, 6], wl_d, "wlt", dt=F32R)
            b0t = cload([128, 4], b0_d, "b0t")
            b1t = cload([128, 4], b1_d, "b1t")
            b2t = cload([128, 4], b2_d, "b2t")
            blt = cload([3, 1], bl_d, "blt")

            for blk in range(nblk):
                bs = slice(blk * BLK, (blk + 1) * BLK)
                a2 = [bp.tile([128, BLK], F32R, tag=f"a2_{k}", name=f"a2_{k}") for k in range(2)]
                outb = bp.tile([3, BLK], F32, tag="outb", name="outb", bufs=1)
                e0loc = blk // 4

                # ---- stage 1: per 512-px chunk: posenc -> L0 -> L1 -> a2 ----
                for c in range(CPB):
                    cs = slice(c * CHUNK, (c + 1) * CHUNK)
                    gs = slice(blk * BLK + c * CHUNK, blk * BLK + (c + 1) * CHUNK)
                    e1 = (blk % 4) * 4 + c // 2
                    # rows: 0:26 sin-angles, 32:58 cos-angles, 64:66 xy,
                    # gaps zero -- matching zero rows in w0e. Sin in-place.
                    pe = wp.tile([66, CHUNK], F32R, tag="pe", name="pe")
                    nc.sync.dma_start(pe[:, :], axy_d[:, gs])
                    nc.scalar.activation(pe[0:26, :], pe[0:26, :], AF.Sin)
                    nc.scalar.activation(pe[32:58, :], pe[32:58, :], AF.Sin)

                    a1 = [wp.tile([128, CHUNK], F32R, tag=f"a1_{m}", name=f"a1_{m}") for m in range(2)]
                    for m in range(2):
                        ps0 = pp.tile([128, CHUNK], F32, tag="l0", name="ps0")
                        col = e0loc * 256 + m * 128
                        nc.tensor.matmul(
                            ps0[:, :], w0t[:, col:col + 128], pe[:, :],
                            start=True, stop=True,
                        )
                        # lrelu(v+b) = relu(0.8v + 0.8b) + 0.2(v+b)
                        r0 = wp.tile([128, CHUNK], F32, tag="r0", name="r0")
                        u0 = wp.tile([128, CHUNK], F32, tag="u0", name="u0")
                        nc.scalar.activation(r0[:, :], ps0[:, :], AF.Relu,
                                             bias=b0t[:, 2 + m:3 + m], scale=0.8)
                        nc.vector.tensor_scalar(u0[:, :], ps0[:, :],
                                                b0t[:, m:m + 1], 0.2, ALU.add, ALU.mult)
                        nc.gpsimd.tensor_add(a1[m][:, :], u0[:, :], r0[:, :])

                    for m in range(2):
                        ps1 = pp.tile([128, CHUNK], F32, tag="l1", name="ps1")
                        for k in range(2):
                            col = (e1 * 2 + k) * 256 + m * 128
                            nc.tensor.matmul(
                                ps1[:, :], w1t[:, col:col + 128], a1[k][:, :],
                                start=(k == 0), stop=(k == 1),
                            )
                        r1 = wp.tile([128, CHUNK], F32, tag="t1", name="t1")
                        u1 = wp.tile([128, CHUNK], F32, tag="t2", name="t2")
                        nc.scalar.activation(r1[:, :], ps1[:, :], AF.Relu,
                                             bias=b1t[:, 2 + m:3 + m], scale=0.8)
                        nc.vector.tensor_scalar(u1[:, :], ps1[:, :],
                                                b1t[:, m:m + 1], 0.2, ALU.add, ALU.mult)
                        nc.gpsimd.tensor_add(a2[m][:, cs], u1[:, :], r1[:, :])

                # ---- stage 2: per idx2 expert: L2 -> last layer -> out ----
                for e2 in range(NEXP):
                    a2ap = [
                        a2[k][:, :].rearrange("p (a b t) -> p a b t", a=4, b=16)[:, :, e2, :]
                        for k in range(2)
                    ]
                    a3 = [wp.tile([128, 4, 64], F32R, tag=f"a3_{m}", name=f"a3_{m}") for m in range(2)]
                    for m in range(2):
                        ps2 = pp.tile([128, 4, 64], F32, tag="l2", name="ps2")
                        for k in range(2):
                            col = (e2 * 2 + k) * 256 + m * 128
                            nc.tensor.matmul(
                                ps2[:, :, :], w2t[:, col:col + 128], a2ap[k],
                                start=(k == 0), stop=(k == 1),
                            )
                        r2 = wp.tile([128, 4, 64], F32, tag="r2", name="r2")
                        u2 = wp.tile([128, 4, 64], F32, tag="u2", name="u2")
                        nc.scalar.activation(r2[:, :, :], ps2[:, :, :], AF.Relu,
                                             bias=b2t[:, 2 + m:3 + m], scale=0.8)
                        nc.vector.tensor_scalar(u2[:, :, :], ps2[:, :, :],
                                                b2t[:, m:m + 1], 0.2, ALU.add, ALU.mult)
                        nc.gpsimd.tensor_add(a3[m][:, :, :], u2[:, :, :], r2[:, :, :])
                    psl = pp.tile([3, 4, 64], F32, tag="last", name="psl")
                    for k in range(2):
                        nc.tensor.matmul(
                            psl[:, :, :], wlt[:, 3 * k:3 * k + 3], a3[k][:, :, :],
                            start=(k == 0), stop=(k == 1),
                        )
                    obap = outb[:, :].rearrange(
                        "p (a b t) -> p a b t", a=4, b=16)[:, :, e2, :]
                    nc.vector.tensor_scalar_add(obap, psl[:, :, :], blt[:, 0:1])

                nc.sync.dma_start(out_d[:, bs], outb[:, :])

    nc.finalize()
    return nc


def _routing(x, labels):
    """Exact replica of reference normalization + per-layer tile indices."""
    x = np.asarray(x, np.float32)
    labels = np.asarray(labels, np.float32)
    xn = ((x / labels).astype(np.float32) - np.float32(0.5)) * np.float32(2.0)
    xi = (xn - np.float32(0.5)) * np.float32(2.0)
    idxs = []
    cum = np.ones(2, np.float32)
    for td in ((4, 4), (4, 4), (4, 4)):
        tda = np.asarray(td, np.float32)
        cum = cum * tda
        ax = np.floor(xi * cum) % tda
        idxs.append((ax[:, 0] * td[1] + ax[:, 1]).astype(np.int64))
    return xn, idxs


def _prepare(inputs, nblk=NBLK_FULL):
    """Host-side: sort, shard, and pack per-core device inputs."""
    x = np.asarray(inputs["x"], np.float32)
    labels = np.asarray(inputs["labels"], np.float32)
    w0 = np.asarray(inputs["w0"], np.float32)
    b0 = np.asarray(inputs["b0"], np.float32)
    w1 = np.asarray(inputs["w1"], np.float32)
    b1 = np.asarray(inputs["b1"], np.float32)
    w2 = np.asarray(inputs["w2"], np.float32)
    b2 = np.asarray(inputs["b2"], np.float32)
    w_last = np.asarray(inputs["w_last"], np.float32)
    b_last = np.asarray(inputs["b_last"], np.float32)

    n = x.shape[0]
    assert n == NPIX, f"expected {NPIX} pixels, got {n}"
    npc = nblk * BLK

    xn, idxs = _routing(x, labels)
    key = idxs[0] * 256 + idxs[1] * 16 + idxs[2]
    perm = np.argsort(key, kind="stable")

    # positional-encoding angles, fp32-exact like the reference, then
    # range-reduced to [-pi, pi) in f64 so HW Sin stays accurate
    freqs = (np.float32(np.pi) * np.float32(2.0) ** np.arange(NFREQ, dtype=np.float32)).astype(np.float32)
    ang = (xn[:, None, :] * freqs[None, :, None]).astype(np.float32).reshape(n, 26)
    a64 = ang.astype(np.float64)
    psis = ((a64 + np.pi) % (2 * np.pi) - np.pi).astype(np.float32)
    psic = ((a64 + np.pi / 2 + np.pi) % (2 * np.pi) - np.pi).astype(np.float32)

    sin_rows = [2 + 4 * f + 2 * d for f in range(NFREQ) for d in range(2)]
    cos_rows = [r + 1 for r in sin_rows]

    w1e = np.ascontiguousarray(
        w1.reshape(NEXP, 2, 128, DH).transpose(2, 0, 1, 3).reshape(128, NEXP * 2 * DH))
    w2e = np.ascontiguousarray(
        w2.reshape(NEXP, 2, 128, DH).transpose(2, 0, 1, 3).reshape(128, NEXP * 2 * DH))
    wle = np.ascontiguousarray(w_last.reshape(2, 128, 3).transpose(1, 0, 2).reshape(128, 6))
    def packb(b):
        bb = b.reshape(DH).reshape(2, 128).T  # [128, 2]
        return np.ascontiguousarray(np.concatenate([bb, np.float32(0.8) * bb], axis=1))

    b0e = packb(b0)
    b1e = packb(b1)
    b2e = packb(b2)
    ble = np.ascontiguousarray(b_last.reshape(3, 1))

    j = np.arange(npc)
    i1 = (j // 1024) % 16
    i2 = (j // 64) % 16

    in_maps = []
    for c in range(NCORES):
        pc = perm[c * NPC:c * NPC + npc]
        kc = key[pc]
        eg = [int(kc[0] // 256), int(kc[min(NPC // 2, npc - 1)] // 256)]
        expected = np.repeat(np.array(eg, np.int64), NPC // 2)[:npc] * 256 + i1 * 16 + i2
        assert np.array_equal(kc, expected), f"core {c}: tile layout mismatch"

        axy = np.zeros((66, npc), np.float32)
        axy[0:26] = psis[pc].T
        axy[32:58] = psic[pc].T
        axy[64:66] = xn[pc].T

        def pack_w0(e):
            we = np.zeros((66, 256), np.float32)
            we[0:26] = w0[e][sin_rows]
            we[32:58] = w0[e][cos_rows]
            we[64:66] = w0[e][[0, 1]]
            return we

        w0e = np.concatenate([pack_w0(e) for e in eg], axis=1)
        in_maps.append({
            "axy": np.ascontiguousarray(axy),
            "w0e": np.ascontiguousarray(w0e),
            "w1e": w1e, "w2e": w2e, "wle": wle,
            "b0e": b0e, "b1e": b1e, "b2e": b2e, "ble": ble,
        })
    return in_maps, perm


def kernel(**inputs):
    global LAST_RESULT
    in_maps, perm = _prepare(inputs)
    if "nc" not in _NC_CACHE:
        _NC_CACHE["nc"] = _build_nc()
    nc = _NC_CACHE["nc"]
    global LAST_EXEC_S
    trace = os.environ.get("BASS_KERNEL_TRACE", "0") == "1"
    t0 = time.time()
    try:
        res = run_bass_kernel_spmd(nc, in_maps, list(range(NCORES)), trace=trace)
    except ModuleNotFoundError:  # no NTFF profile hook in this env
        res = run_bass_kernel_spmd(nc, in_maps, list(range(NCORES)), trace=False)
    LAST_EXEC_S = time.time() - t0
    LAST_RESULT = res
    out = np.empty((NPIX, 3), np.float32)
    for c in range(NCORES):
        out[perm[c * NPC:(c + 1) * NPC]] = res.results[c]["out"].T
    return out
